# revision 7
# baseline (speedup 1.0000x reference)
"""AttentionBlock (GroupNorm + single-head spatial attention + SE gate + residual)
Trainium2 Bass/Tile kernel, data-parallel over batch across 8 NeuronCores.

Full shapes: x [32, 256, 32, 32] f32 -> out [32, 256, 32, 32] f32.
Per core: 4 samples. Per sample (C=256, N=1024):
  xn = GroupNorm(x) (32 groups)            [C, N]  (bf16)
  q, k = Wqk @ xn                          [2C, N] (bf16, [c,n] layout)
  vT = xn^T @ WvT                          [N, C]  (bf16, [n,c] layout - direct!)
  esT = exp((k^T q) / 16)                  [N, N]  ([j, i] layout, j = softmax axis)
  sums_bc = ones128 @ esT  (accum over j)  [128, N] (each row = sum_j exp)
  r = 1/sums (reciprocal_approx_fast)      [128, N]
  xat = (vT^T @ esT) * r                   [C, N]  (unnormalized AV, scaled after)
  y = Wp @ xat                             [C, N]
  out = x + (y + bp) * gate[c]             (gate = SE sigmoid path from channel means)

No transposes anywhere: softmax reductions over j land on the PE contraction
axis (ones-matmul), the normalization is a rank-1 column scale folded in after
the AV matmul.
"""

import numpy as np
import ml_dtypes

B, C, HW, N = 32, 256, 32, 1024
NCORES = 8
BL = B // NCORES          # samples per core
GROUPS = 32
GSIZE = C // GROUPS       # 8 channels per group
EPS = 1e-5
CT = 2                    # channel partition tiles (256 = 2*128)
P = 128

_CACHE = {}


def _build_program(want_bias_v):
    import concourse.bacc as bacc
    import concourse.mybir as mybir
    import concourse.tile as tile

    f32 = mybir.dt.float32
    bf16 = mybir.dt.bfloat16
    AX = mybir.AxisListType.X
    AF = mybir.ActivationFunctionType
    ALU = mybir.AluOpType

    nc = bacc.Bacc()

    # ---- DRAM I/O ----
    x_d = nc.dram_tensor("x", [BL, C, N], f32, kind="ExternalInput")
    out_d = nc.dram_tensor("out", [BL, C, N], f32, kind="ExternalOutput")
    wqk_d = nc.dram_tensor("wqk", [P, 2, 512], bf16, kind="ExternalInput")
    wv_d = nc.dram_tensor("wv", [P, 2, C], bf16, kind="ExternalInput")
    wp_d = nc.dram_tensor("wp", [P, 2, C], bf16, kind="ExternalInput")
    w1_d = nc.dram_tensor("w1", [P, 2, 64], f32, kind="ExternalInput")
    w2_d = nc.dram_tensor("w2", [64, C], f32, kind="ExternalInput")
    gamma_d = nc.dram_tensor("gamma", [P, 2], f32, kind="ExternalInput")
    beta_d = nc.dram_tensor("beta", [P, 2], f32, kind="ExternalInput")
    bqk_d = nc.dram_tensor("bqk", [P, 4], f32, kind="ExternalInput")
    bv_d = nc.dram_tensor("bv", [P, 2], f32, kind="ExternalInput")
    bp_d = nc.dram_tensor("bp", [P, 2], f32, kind="ExternalInput")
    b1_d = nc.dram_tensor("b1", [64, 1], f32, kind="ExternalInput")
    b2_d = nc.dram_tensor("b2", [P, 2], f32, kind="ExternalInput")
    gm_d = nc.dram_tensor("gm", [P, 16], f32, kind="ExternalInput")
    gmt_d = nc.dram_tensor("gmt", [16, P], f32, kind="ExternalInput")
    ones_d = nc.dram_tensor("ones", [P, P], bf16, kind="ExternalInput")

    with tile.TileContext(nc) as tc:
        with (
            tc.tile_pool(name="persist", bufs=1) as persist,
            tc.tile_pool(name="qk", bufs=2) as qk_pool,
            tc.tile_pool(name="vt", bufs=2) as vt_pool,
            tc.tile_pool(name="es", bufs=2) as es_pool,
            tc.tile_pool(name="xat", bufs=2) as xat_pool,
            tc.tile_pool(name="rr", bufs=2) as r_pool,
            tc.tile_pool(name="junk", bufs=2) as junk_pool,
            tc.tile_pool(name="outp", bufs=3) as out_pool,
            tc.tile_pool(name="psb", bufs=3, space="PSUM") as psum_big,
            tc.tile_pool(name="pss", bufs=2, space="PSUM") as psum_small,
        ):
            # ---- load constants / weights ----
            wqk_sb = persist.tile([P, 2, 512], bf16)
            nc.sync.dma_start(out=wqk_sb, in_=wqk_d[:, :, :])
            wv_sb = persist.tile([P, 2, C], bf16)
            nc.sync.dma_start(out=wv_sb, in_=wv_d[:, :, :])
            wp_sb = persist.tile([P, 2, C], bf16)
            nc.sync.dma_start(out=wp_sb, in_=wp_d[:, :, :])
            w1_sb = persist.tile([P, 2, 64], f32)
            nc.sync.dma_start(out=w1_sb, in_=w1_d[:, :, :])
            w2_sb = persist.tile([64, C], f32)
            nc.sync.dma_start(out=w2_sb, in_=w2_d[:, :])
            gamma_sb = persist.tile([P, 2], f32)
            nc.sync.dma_start(out=gamma_sb, in_=gamma_d[:, :])
            beta_sb = persist.tile([P, 2], f32)
            nc.sync.dma_start(out=beta_sb, in_=beta_d[:, :])
            bqk_sb = persist.tile([P, 4], f32)
            nc.sync.dma_start(out=bqk_sb, in_=bqk_d[:, :])
            bv_sb = persist.tile([P, 2], f32)
            nc.sync.dma_start(out=bv_sb, in_=bv_d[:, :])
            bp_sb = persist.tile([P, 2], f32)
            nc.sync.dma_start(out=bp_sb, in_=bp_d[:, :])
            b1_sb = persist.tile([64, 1], f32)
            nc.sync.dma_start(out=b1_sb, in_=b1_d[:, :])
            b2_sb = persist.tile([P, 2], f32)
            nc.sync.dma_start(out=b2_sb, in_=b2_d[:, :])
            gm_sb = persist.tile([P, 16], f32)
            nc.sync.dma_start(out=gm_sb, in_=gm_d[:, :])
            gmt_sb = persist.tile([16, P], f32)
            nc.sync.dma_start(out=gmt_sb, in_=gmt_d[:, :])
            ones_sb = persist.tile([P, P], bf16)
            nc.sync.dma_start(out=ones_sb, in_=ones_d[:, :])

            # ---- load x in [c, b, n] layout ----
            x_sb = persist.tile([P, CT, BL, N], f32)
            for ct in range(CT):
                nc.sync.dma_start(
                    out=x_sb[:, ct],
                    in_=x_d[:, ct * P:(ct + 1) * P, :].transpose([1, 0, 2]),
                )

            # ---- GroupNorm stats + SE pooled path ----
            sums_c = persist.tile([P, CT, BL], f32)    # per-channel sums
            sumsq_c = persist.tile([P, CT, BL], f32)   # per-channel sum of squares
            for ct in range(CT):
                for b in range(BL):
                    nc.vector.reduce_sum(
                        out=sums_c[:, ct, b:b + 1], in_=x_sb[:, ct, b], axis=AX)
                    jt = junk_pool.tile([P, N], bf16, tag="junk")
                    nc.scalar.activation(
                        out=jt, in_=x_sb[:, ct, b], func=AF.Square,
                        accum_out=sumsq_c[:, ct, b:b + 1])

            # group stats via group-mask matmuls (fp32, tiny)
            eps_sb = persist.tile([16, 1], f32)
            nc.vector.memset(eps_sb, EPS)
            a_sb = persist.tile([P, CT, BL], f32)   # per-channel scale
            bb_sb = persist.tile([P, CT, BL], f32)  # per-channel offset
            for ct in range(CT):
                ps_g = psum_small.tile([16, 2 * BL], f32, tag="pss")
                nc.tensor.matmul(ps_g[:, 0:BL], gm_sb, sums_c[:, ct],
                                 start=True, stop=True)
                nc.tensor.matmul(ps_g[:, BL:2 * BL], gm_sb, sumsq_c[:, ct],
                                 start=True, stop=True)
                nmean = persist.tile([16, BL], f32)
                nc.vector.tensor_scalar_mul(nmean, ps_g[:, 0:BL],
                                            -1.0 / (GSIZE * N))
                ex2 = persist.tile([16, BL], f32)
                nc.vector.tensor_scalar_mul(ex2, ps_g[:, BL:2 * BL],
                                            1.0 / (GSIZE * N))
                var = persist.tile([16, BL], f32)
                nc.vector.tensor_mul(var, nmean, nmean)
                nc.vector.tensor_sub(var, ex2, var)
                sd = persist.tile([16, BL], f32)
                nc.scalar.activation(out=sd, in_=var, func=AF.Sqrt, bias=eps_sb)
                rsm = persist.tile([16, 2 * BL], f32)
                nc.vector.reciprocal(rsm[:, 0:BL], sd)
                nc.vector.tensor_mul(rsm[:, BL:2 * BL], nmean, rsm[:, 0:BL])
                ps_bc = psum_small.tile([P, 2 * BL], f32, tag="pss")
                nc.tensor.matmul(ps_bc, gmt_sb, rsm, start=True, stop=True)
                nc.vector.tensor_scalar_mul(a_sb[:, ct], ps_bc[:, 0:BL],
                                            gamma_sb[:, ct:ct + 1])
                nc.vector.tensor_scalar(
                    out=bb_sb[:, ct], in0=ps_bc[:, BL:2 * BL],
                    scalar1=gamma_sb[:, ct:ct + 1],
                    scalar2=beta_sb[:, ct:ct + 1],
                    op0=ALU.mult, op1=ALU.add)

            # ---- normalize -> xn (bf16) ----
            xn_sb = persist.tile([P, CT, BL, N], bf16)
            for ct in range(CT):
                for b in range(BL):
                    nc.scalar.activation(
                        out=xn_sb[:, ct, b], in_=x_sb[:, ct, b],
                        func=AF.Identity,
                        bias=bb_sb[:, ct, b:b + 1], scale=a_sb[:, ct, b:b + 1])

            # ---- SE gate (per core, channel means already in sums_c) ----
            ps_h1 = psum_small.tile([64, BL], f32, tag="pss")
            for ct in range(CT):
                nc.tensor.matmul(ps_h1, w1_sb[:, ct], sums_c[:, ct],
                                 start=(ct == 0), stop=(ct == 1))
            h1_sb = persist.tile([64, BL], f32)
            nc.scalar.activation(out=h1_sb, in_=ps_h1, func=AF.Relu,
                                 bias=b1_sb[:, 0:1], scale=1.0 / N)
            gate_sb = persist.tile([P, CT, BL], f32)
            for ot in range(CT):
                ps_gate = psum_small.tile([P, BL], f32, tag="pss")
                nc.tensor.matmul(ps_gate, w2_sb[:, ot * P:(ot + 1) * P], h1_sb,
                                 start=True, stop=True)
                nc.scalar.activation(out=gate_sb[:, ot], in_=ps_gate,
                                     func=AF.Sigmoid, bias=b2_sb[:, ot:ot + 1])

            # ---- per-sample attention ----
            for b in range(BL):
                # q, k : [c, n] layout. m-tile 0,1 = q rows; 2,3 = k rows
                qk_sb = qk_pool.tile([P, 4, N], bf16, tag="qk")
                for m in range(4):
                    ps_qk = psum_big.tile([P, N], f32, tag="psb")
                    for ns in range(2):
                        for kt in range(CT):
                            nc.tensor.matmul(
                                ps_qk[:, ns * 512:(ns + 1) * 512],
                                wqk_sb[:, kt, m * P:(m + 1) * P],
                                xn_sb[:, kt, b, ns * 512:(ns + 1) * 512],
                                start=(kt == 0), stop=(kt == 1))
                    nc.vector.tensor_scalar_add(qk_sb[:, m], ps_qk,
                                                bqk_sb[:, m:m + 1])

                # vT : [n, c] layout (j on partitions)
                vt_sb = vt_pool.tile([P, 8, C], bf16, tag="vt")
                for jt in range(8):
                    ps_vt = psum_small.tile([P, C], f32, tag="pss")
                    for kt in range(CT):
                        nc.tensor.matmul(
                            ps_vt,
                            xn_sb[:, kt, b, jt * P:(jt + 1) * P],
                            wv_sb[:, kt],
                            start=(kt == 0), stop=(kt == 1))
                    nc.scalar.activation(out=vt_sb[:, jt], in_=ps_vt,
                                         func=AF.Copy)

                # esT = exp(S^T / 16) : [j, i] layout
                es_sb = es_pool.tile([P, 8, N], bf16, tag="es")
                for mt in range(8):
                    ps_s = psum_big.tile([P, N], f32, tag="psb")
                    for ns in range(2):
                        for kt in range(CT):
                            nc.tensor.matmul(
                                ps_s[:, ns * 512:(ns + 1) * 512],
                                qk_sb[:, 2 + kt, mt * P:(mt + 1) * P],
                                qk_sb[:, kt, ns * 512:(ns + 1) * 512],
                                start=(kt == 0), stop=(kt == 1))
                    nc.scalar.activation(out=es_sb[:, mt], in_=ps_s,
                                         func=AF.Exp, scale=0.0625)

                # softmax denominators, broadcast to 128 partitions
                ps_sum = psum_big.tile([P, N], f32, tag="psb")
                for ns in range(2):
                    for jt in range(8):
                        nc.tensor.matmul(
                            ps_sum[:, ns * 512:(ns + 1) * 512],
                            ones_sb,
                            es_sb[:, jt, ns * 512:(ns + 1) * 512],
                            start=(jt == 0), stop=(jt == 7))
                r_sb = r_pool.tile([P, N], f32, tag="rr")
                nc.vector.reciprocal_approx_fast(out=r_sb, in_=ps_sum)

                # AV (unnormalized) then column-scale by r
                xat_sb = xat_pool.tile([P, CT, N], bf16, tag="xat")
                for ct2 in range(CT):
                    ps_av = psum_big.tile([P, N], f32, tag="psb")
                    for ns in range(2):
                        for jt in range(8):
                            nc.tensor.matmul(
                                ps_av[:, ns * 512:(ns + 1) * 512],
                                vt_sb[:, jt, ct2 * P:(ct2 + 1) * P],
                                es_sb[:, jt, ns * 512:(ns + 1) * 512],
                                start=(jt == 0), stop=(jt == 7))
                    if want_bias_v:
                        tmp = r_pool.tile([P, N], f32, tag="avtmp")
                        nc.vector.tensor_mul(tmp, ps_av, r_sb)
                        nc.vector.tensor_scalar_add(xat_sb[:, ct2], tmp,
                                                    bv_sb[:, ct2:ct2 + 1])
                    else:
                        nc.vector.tensor_mul(xat_sb[:, ct2], ps_av, r_sb)

                # proj + SE gate + residual
                for ot in range(CT):
                    ps_y = psum_big.tile([P, N], f32, tag="psb")
                    for ns in range(2):
                        for kt2 in range(CT):
                            nc.tensor.matmul(
                                ps_y[:, ns * 512:(ns + 1) * 512],
                                wp_sb[:, kt2, ot * P:(ot + 1) * P],
                                xat_sb[:, kt2, ns * 512:(ns + 1) * 512],
                                start=(kt2 == 0), stop=(kt2 == 1))
                    out_t = out_pool.tile([P, N], f32, tag="outp")
                    nc.vector.tensor_scalar(
                        out=out_t, in0=ps_y,
                        scalar1=bp_sb[:, ot:ot + 1],
                        scalar2=gate_sb[:, ot, b:b + 1],
                        op0=ALU.add, op1=ALU.mult)
                    nc.vector.tensor_add(out_t, out_t, x_sb[:, ot, b])
                    nc.sync.dma_start(
                        out=out_d[b, ot * P:(ot + 1) * P, :], in_=out_t)

    nc.compile()
    return nc


def _prep_inputs(x, gn_gamma, gn_beta, w_qkv, b_qkv, w_proj, b_proj,
                 w_se1, b_se1, w_se2, b_se2):
    bf = ml_dtypes.bfloat16
    f32 = np.float32

    def pt(w):  # [K, M] -> [128, K//128, M] partition-tiled
        K, M = w.shape
        return np.ascontiguousarray(w.reshape(K // P, P, M).transpose(1, 0, 2))

    wqk = pt(np.ascontiguousarray(w_qkv[:512].T)).astype(bf)       # [128,2,512]
    wv = pt(np.ascontiguousarray(w_qkv[512:].T)).astype(bf)        # [128,2,256]
    wp = pt(np.ascontiguousarray(w_proj.T)).astype(bf)             # [128,2,256]
    w1 = pt(np.ascontiguousarray(w_se1.T)).astype(f32)             # [128,2,64]
    w2 = np.ascontiguousarray(w_se2.T).astype(f32)                 # [64,256]

    def pcol(v):  # [256] -> [128, 2]
        return np.ascontiguousarray(v.reshape(2, P).T).astype(f32)

    gm = np.zeros((P, 16), f32)
    gm[np.arange(P), np.arange(P) // GSIZE] = 1.0
    shared = {
        "wqk": wqk, "wv": wv, "wp": wp, "w1": w1, "w2": w2,
        "gamma": pcol(gn_gamma), "beta": pcol(gn_beta),
        "bqk": np.ascontiguousarray(b_qkv[:512].reshape(4, P).T).astype(f32),
        "bv": pcol(b_qkv[512:]), "bp": pcol(b_proj),
        "b1": np.asarray(b_se1, f32).reshape(64, 1),
        "b2": pcol(b_se2),
        "gm": gm, "gmt": np.ascontiguousarray(gm.T),
        "ones": np.ones((P, P), bf),
    }
    xr = np.asarray(x, f32).reshape(B, C, N)
    in_maps = []
    for i in range(NCORES):
        m = dict(shared)
        m["x"] = np.ascontiguousarray(xr[i * BL:(i + 1) * BL])
        in_maps.append(m)
    want_bias_v = bool(np.any(np.asarray(b_qkv[512:]) != 0))
    return in_maps, want_bias_v


def _get_program(want_bias_v):
    key = ("prog", want_bias_v)
    if key not in _CACHE:
        _CACHE[key] = _build_program(want_bias_v)
    return _CACHE[key]


def run(inputs, trace=False, trace_kwargs=None):
    """Build + run on all 8 cores. Returns (full_out, BassKernelResults)."""
    from concourse.bass_utils import run_bass_kernel_spmd

    in_maps, want_bias_v = _prep_inputs(**inputs)
    nc = _get_program(want_bias_v)
    kw = {}
    if trace:
        kw["trace"] = True
        if trace_kwargs:
            kw["trace_kwargs"] = trace_kwargs
    res = run_bass_kernel_spmd(nc, in_maps, list(range(NCORES)), **kw)
    out = np.concatenate([res.results[i]["out"] for i in range(NCORES)], axis=0)
    return out.reshape(B, C, HW, HW).astype(np.float32), res


def kernel(**inputs):
    out, _ = run(inputs, trace=False)
    return out


# revision 10
# speedup vs baseline: 1.0377x; 1.0377x over previous
"""AttentionBlock (GroupNorm + single-head spatial attention + SE gate + residual)
Trainium2 Bass/Tile kernel, data-parallel over batch across 8 NeuronCores.

Full shapes: x [32, 256, 32, 32] f32 -> out [32, 256, 32, 32] f32.
Per core: 4 samples. Per sample (C=256, N=1024):
  xn = GroupNorm(x) (32 groups)            [C, N]  (bf16)
  q, k = Wqk @ xn                          [2C, N] (bf16, [c,n] layout)
  vT = xn^T @ WvT                          [N, C]  (bf16, [n,c] layout - direct!)
  esT = exp((k^T q) / 16)                  [N, N]  ([j, i] layout, j = softmax axis)
  sums_bc = ones128 @ esT  (accum over j)  [128, N] (each row = sum_j exp)
  r = 1/sums (reciprocal_approx_fast)      [128, N]
  xat = (vT^T @ esT) * r                   [C, N]  (unnormalized AV, scaled after)
  y = Wp @ xat                             [C, N]
  out = x + (y + bp) * gate[c]             (gate = SE sigmoid path from channel means)

No transposes anywhere: softmax reductions over j land on the PE contraction
axis (ones-matmul), the normalization is a rank-1 column scale folded in after
the AV matmul.
"""

import numpy as np
import ml_dtypes

B, C, HW, N = 32, 256, 32, 1024
NCORES = 8
BL = B // NCORES          # samples per core
GROUPS = 32
GSIZE = C // GROUPS       # 8 channels per group
EPS = 1e-5
CT = 2                    # channel partition tiles (256 = 2*128)
P = 128

_CACHE = {}


def _build_program(want_bias_v):
    import concourse.bacc as bacc
    import concourse.mybir as mybir
    import concourse.tile as tile

    f32 = mybir.dt.float32
    bf16 = mybir.dt.bfloat16
    AX = mybir.AxisListType.X
    AF = mybir.ActivationFunctionType
    ALU = mybir.AluOpType

    nc = bacc.Bacc()

    # ---- DRAM I/O ----
    x_d = nc.dram_tensor("x", [BL, C, N], f32, kind="ExternalInput")
    out_d = nc.dram_tensor("out", [BL, C, N], f32, kind="ExternalOutput")
    wqk_d = nc.dram_tensor("wqk", [P, 2, 512], bf16, kind="ExternalInput")
    wv_d = nc.dram_tensor("wv", [P, 2, C], bf16, kind="ExternalInput")
    wp_d = nc.dram_tensor("wp", [P, 2, C], bf16, kind="ExternalInput")
    w1_d = nc.dram_tensor("w1", [P, 2, 64], f32, kind="ExternalInput")
    w2_d = nc.dram_tensor("w2", [64, C], f32, kind="ExternalInput")
    gamma_d = nc.dram_tensor("gamma", [P, 2], f32, kind="ExternalInput")
    beta_d = nc.dram_tensor("beta", [P, 2], f32, kind="ExternalInput")
    bqk_d = nc.dram_tensor("bqk", [P, 4], f32, kind="ExternalInput")
    bv_d = nc.dram_tensor("bv", [P, 2], f32, kind="ExternalInput")
    bp_d = nc.dram_tensor("bp", [P, 2], f32, kind="ExternalInput")
    b1_d = nc.dram_tensor("b1", [64, 1], f32, kind="ExternalInput")
    b2_d = nc.dram_tensor("b2", [P, 2], f32, kind="ExternalInput")
    gm_d = nc.dram_tensor("gm", [P, 16], f32, kind="ExternalInput")
    gmt_d = nc.dram_tensor("gmt", [16, P], f32, kind="ExternalInput")
    ones_d = nc.dram_tensor("ones", [P, P], bf16, kind="ExternalInput")

    with tile.TileContext(nc) as tc:
        with (
            tc.tile_pool(name="persist", bufs=1) as persist,
            tc.tile_pool(name="qk", bufs=2) as qk_pool,
            tc.tile_pool(name="vt", bufs=2) as vt_pool,
            tc.tile_pool(name="es", bufs=2) as es_pool,
            tc.tile_pool(name="xat", bufs=2) as xat_pool,
            tc.tile_pool(name="rr", bufs=2) as r_pool,
            tc.tile_pool(name="junk", bufs=2) as junk_pool,
            tc.tile_pool(name="outp", bufs=3) as out_pool,
            tc.tile_pool(name="psb", bufs=3, space="PSUM") as psum_big,
            tc.tile_pool(name="pss", bufs=2, space="PSUM") as psum_small,
        ):
            # ---- load constants / weights ----
            wqk_sb = persist.tile([P, 2, 512], bf16)
            nc.sync.dma_start(out=wqk_sb, in_=wqk_d[:, :, :])
            wv_sb = persist.tile([P, 2, C], bf16)
            nc.sync.dma_start(out=wv_sb, in_=wv_d[:, :, :])
            wp_sb = persist.tile([P, 2, C], bf16)
            nc.sync.dma_start(out=wp_sb, in_=wp_d[:, :, :])
            w1_sb = persist.tile([P, 2, 64], f32)
            nc.sync.dma_start(out=w1_sb, in_=w1_d[:, :, :])
            w2_sb = persist.tile([64, C], f32)
            nc.sync.dma_start(out=w2_sb, in_=w2_d[:, :])
            gamma_sb = persist.tile([P, 2], f32)
            nc.sync.dma_start(out=gamma_sb, in_=gamma_d[:, :])
            beta_sb = persist.tile([P, 2], f32)
            nc.sync.dma_start(out=beta_sb, in_=beta_d[:, :])
            bqk_sb = persist.tile([P, 4], f32)
            nc.sync.dma_start(out=bqk_sb, in_=bqk_d[:, :])
            bv_sb = persist.tile([P, 2], f32)
            nc.sync.dma_start(out=bv_sb, in_=bv_d[:, :])
            bp_sb = persist.tile([P, 2], f32)
            nc.sync.dma_start(out=bp_sb, in_=bp_d[:, :])
            b1_sb = persist.tile([64, 1], f32)
            nc.sync.dma_start(out=b1_sb, in_=b1_d[:, :])
            b2_sb = persist.tile([P, 2], f32)
            nc.sync.dma_start(out=b2_sb, in_=b2_d[:, :])
            gm_sb = persist.tile([P, 16], f32)
            nc.sync.dma_start(out=gm_sb, in_=gm_d[:, :])
            gmt_sb = persist.tile([16, P], f32)
            nc.sync.dma_start(out=gmt_sb, in_=gmt_d[:, :])
            ones_sb = persist.tile([P, P], bf16)
            nc.sync.dma_start(out=ones_sb, in_=ones_d[:, :])

            eps_sb = persist.tile([16, 1], f32)
            nc.vector.memset(eps_sb, EPS)

            # ---- x load + GroupNorm stats, pipelined per (sample, ctile) ----
            # GroupNorm scale/offset chains run vectorized over sample PAIRS so
            # sample 0's normalize only waits on samples 0-1, not all four.
            x_sb = persist.tile([P, CT, BL, N], f32)
            sums_c = persist.tile([P, CT, BL], f32)    # per-channel sums
            sumsq_c = persist.tile([P, CT, BL], f32)   # per-channel sum sq
            a_sb = persist.tile([P, CT, BL], f32)      # per-channel scale
            bb_sb = persist.tile([P, CT, BL], f32)     # per-channel offset
            for b in range(BL):
                for ct in range(CT):
                    nc.sync.dma_start(out=x_sb[:, ct, b],
                                      in_=x_d[b, ct * P:(ct + 1) * P, :])
                for ct in range(CT):
                    nc.vector.reduce_sum(
                        out=sums_c[:, ct, b:b + 1], in_=x_sb[:, ct, b], axis=AX)
                    jt = junk_pool.tile([P, N], bf16, tag="junk")
                    nc.scalar.activation(
                        out=jt, in_=x_sb[:, ct, b], func=AF.Square,
                        accum_out=sumsq_c[:, ct, b:b + 1])
                if b % 2 == 1:
                    pr = slice(b - 1, b + 1)  # this sample pair
                    for ct in range(CT):
                        ps_g = psum_small.tile([16, 4], f32, tag="pss")
                        nc.tensor.matmul(ps_g[:, 0:2], gm_sb,
                                         sums_c[:, ct, pr],
                                         start=True, stop=True)
                        nc.tensor.matmul(ps_g[:, 2:4], gm_sb,
                                         sumsq_c[:, ct, pr],
                                         start=True, stop=True)
                        nmean = persist.tile([16, 2], f32)
                        nc.vector.tensor_scalar_mul(nmean, ps_g[:, 0:2],
                                                    -1.0 / (GSIZE * N))
                        var = persist.tile([16, 2], f32)
                        nc.vector.tensor_scalar_mul(var, ps_g[:, 2:4],
                                                    1.0 / (GSIZE * N))
                        msq = persist.tile([16, 2], f32)
                        nc.vector.tensor_mul(msq, nmean, nmean)
                        nc.vector.tensor_sub(var, var, msq)
                        sd = persist.tile([16, 2], f32)
                        nc.scalar.activation(out=sd, in_=var, func=AF.Sqrt,
                                             bias=eps_sb)
                        rsm = persist.tile([16, 4], f32)
                        nc.vector.reciprocal(rsm[:, 0:2], sd)
                        nc.vector.tensor_mul(rsm[:, 2:4], nmean, rsm[:, 0:2])
                        ps_bc = psum_small.tile([P, 4], f32, tag="pss")
                        nc.tensor.matmul(ps_bc, gmt_sb, rsm,
                                         start=True, stop=True)
                        nc.vector.tensor_scalar_mul(
                            a_sb[:, ct, pr], ps_bc[:, 0:2],
                            gamma_sb[:, ct:ct + 1])
                        nc.vector.tensor_scalar(
                            out=bb_sb[:, ct, pr], in0=ps_bc[:, 2:4],
                            scalar1=gamma_sb[:, ct:ct + 1],
                            scalar2=beta_sb[:, ct:ct + 1],
                            op0=ALU.mult, op1=ALU.add)

            # ---- SE gate (per core, channel means already in sums_c) ----
            ps_h1 = psum_small.tile([64, BL], f32, tag="pss")
            for ct in range(CT):
                nc.tensor.matmul(ps_h1, w1_sb[:, ct], sums_c[:, ct],
                                 start=(ct == 0), stop=(ct == 1))
            h1_sb = persist.tile([64, BL], f32)
            nc.scalar.activation(out=h1_sb, in_=ps_h1, func=AF.Relu,
                                 bias=b1_sb[:, 0:1], scale=1.0 / N)
            gate_sb = persist.tile([P, CT, BL], f32)
            for ot in range(CT):
                ps_gate = psum_small.tile([P, BL], f32, tag="pss")
                nc.tensor.matmul(ps_gate, w2_sb[:, ot * P:(ot + 1) * P], h1_sb,
                                 start=True, stop=True)
                nc.scalar.activation(out=gate_sb[:, ot], in_=ps_gate,
                                     func=AF.Sigmoid, bias=b2_sb[:, ot:ot + 1])

            # ---- per-sample attention ----
            xn_sb = persist.tile([P, CT, BL, N], bf16)
            for b in range(BL):
                # normalize -> xn (bf16)
                for ct in range(CT):
                    nc.scalar.activation(
                        out=xn_sb[:, ct, b], in_=x_sb[:, ct, b],
                        func=AF.Identity,
                        bias=bb_sb[:, ct, b:b + 1], scale=a_sb[:, ct, b:b + 1])

                # q, k : [c, n] layout. m-tile 0,1 = q rows; 2,3 = k rows
                qk_sb = qk_pool.tile([P, 4, N], bf16, tag="qk")
                for m in range(4):
                    ps_qk = psum_big.tile([P, N], f32, tag="psb")
                    for ns in range(2):
                        for kt in range(CT):
                            nc.tensor.matmul(
                                ps_qk[:, ns * 512:(ns + 1) * 512],
                                wqk_sb[:, kt, m * P:(m + 1) * P],
                                xn_sb[:, kt, b, ns * 512:(ns + 1) * 512],
                                start=(kt == 0), stop=(kt == 1))
                    nc.vector.tensor_scalar_add(qk_sb[:, m], ps_qk,
                                                bqk_sb[:, m:m + 1])

                # vT : [n, c] layout (j on partitions)
                vt_sb = vt_pool.tile([P, 8, C], bf16, tag="vt")
                for jt in range(8):
                    ps_vt = psum_small.tile([P, C], f32, tag="pss")
                    for kt in range(CT):
                        nc.tensor.matmul(
                            ps_vt,
                            xn_sb[:, kt, b, jt * P:(jt + 1) * P],
                            wv_sb[:, kt],
                            start=(kt == 0), stop=(kt == 1))
                    nc.scalar.activation(out=vt_sb[:, jt], in_=ps_vt,
                                         func=AF.Copy)

                # esT = exp(S^T / 16) : [j, i] layout
                es_sb = es_pool.tile([P, 8, N], bf16, tag="es")
                for mt in range(8):
                    ps_s = psum_big.tile([P, N], f32, tag="psb")
                    for ns in range(2):
                        for kt in range(CT):
                            nc.tensor.matmul(
                                ps_s[:, ns * 512:(ns + 1) * 512],
                                qk_sb[:, 2 + kt, mt * P:(mt + 1) * P],
                                qk_sb[:, kt, ns * 512:(ns + 1) * 512],
                                start=(kt == 0), stop=(kt == 1))
                    nc.scalar.activation(out=es_sb[:, mt], in_=ps_s,
                                         func=AF.Exp, scale=0.0625)

                # softmax denominators, broadcast to 128 partitions
                ps_sum = psum_big.tile([P, N], f32, tag="psb")
                for ns in range(2):
                    for jt in range(8):
                        nc.tensor.matmul(
                            ps_sum[:, ns * 512:(ns + 1) * 512],
                            ones_sb,
                            es_sb[:, jt, ns * 512:(ns + 1) * 512],
                            start=(jt == 0), stop=(jt == 7))
                r_sb = r_pool.tile([P, N], f32, tag="rr")
                nc.vector.reciprocal_approx_fast(out=r_sb, in_=ps_sum)

                # AV (unnormalized) then column-scale by r
                xat_sb = xat_pool.tile([P, CT, N], bf16, tag="xat")
                for ct2 in range(CT):
                    ps_av = psum_big.tile([P, N], f32, tag="psb")
                    for ns in range(2):
                        for jt in range(8):
                            nc.tensor.matmul(
                                ps_av[:, ns * 512:(ns + 1) * 512],
                                vt_sb[:, jt, ct2 * P:(ct2 + 1) * P],
                                es_sb[:, jt, ns * 512:(ns + 1) * 512],
                                start=(jt == 0), stop=(jt == 7))
                    if want_bias_v:
                        tmp = r_pool.tile([P, N], f32, tag="avtmp")
                        nc.vector.tensor_mul(tmp, ps_av, r_sb)
                        nc.vector.tensor_scalar_add(xat_sb[:, ct2], tmp,
                                                    bv_sb[:, ct2:ct2 + 1])
                    else:
                        nc.vector.tensor_mul(xat_sb[:, ct2], ps_av, r_sb)

                # proj + SE gate + residual
                for ot in range(CT):
                    ps_y = psum_big.tile([P, N], f32, tag="psb")
                    for ns in range(2):
                        for kt2 in range(CT):
                            nc.tensor.matmul(
                                ps_y[:, ns * 512:(ns + 1) * 512],
                                wp_sb[:, kt2, ot * P:(ot + 1) * P],
                                xat_sb[:, kt2, ns * 512:(ns + 1) * 512],
                                start=(kt2 == 0), stop=(kt2 == 1))
                    out_t = out_pool.tile([P, N], f32, tag="outp")
                    nc.vector.tensor_scalar(
                        out=out_t, in0=ps_y,
                        scalar1=bp_sb[:, ot:ot + 1],
                        scalar2=gate_sb[:, ot, b:b + 1],
                        op0=ALU.add, op1=ALU.mult)
                    nc.vector.tensor_add(out_t, out_t, x_sb[:, ot, b])
                    nc.sync.dma_start(
                        out=out_d[b, ot * P:(ot + 1) * P, :], in_=out_t)

    nc.compile()
    return nc


def _prep_inputs(x, gn_gamma, gn_beta, w_qkv, b_qkv, w_proj, b_proj,
                 w_se1, b_se1, w_se2, b_se2):
    bf = ml_dtypes.bfloat16
    f32 = np.float32

    def pt(w):  # [K, M] -> [128, K//128, M] partition-tiled
        K, M = w.shape
        return np.ascontiguousarray(w.reshape(K // P, P, M).transpose(1, 0, 2))

    wqk = pt(np.ascontiguousarray(w_qkv[:512].T)).astype(bf)       # [128,2,512]
    wv = pt(np.ascontiguousarray(w_qkv[512:].T)).astype(bf)        # [128,2,256]
    wp = pt(np.ascontiguousarray(w_proj.T)).astype(bf)             # [128,2,256]
    w1 = pt(np.ascontiguousarray(w_se1.T)).astype(f32)             # [128,2,64]
    w2 = np.ascontiguousarray(w_se2.T).astype(f32)                 # [64,256]

    def pcol(v):  # [256] -> [128, 2]
        return np.ascontiguousarray(v.reshape(2, P).T).astype(f32)

    gm = np.zeros((P, 16), f32)
    gm[np.arange(P), np.arange(P) // GSIZE] = 1.0
    shared = {
        "wqk": wqk, "wv": wv, "wp": wp, "w1": w1, "w2": w2,
        "gamma": pcol(gn_gamma), "beta": pcol(gn_beta),
        "bqk": np.ascontiguousarray(b_qkv[:512].reshape(4, P).T).astype(f32),
        "bv": pcol(b_qkv[512:]), "bp": pcol(b_proj),
        "b1": np.asarray(b_se1, f32).reshape(64, 1),
        "b2": pcol(b_se2),
        "gm": gm, "gmt": np.ascontiguousarray(gm.T),
        "ones": np.ones((P, P), bf),
    }
    xr = np.asarray(x, f32).reshape(B, C, N)
    in_maps = []
    for i in range(NCORES):
        m = dict(shared)
        m["x"] = np.ascontiguousarray(xr[i * BL:(i + 1) * BL])
        in_maps.append(m)
    want_bias_v = bool(np.any(np.asarray(b_qkv[512:]) != 0))
    return in_maps, want_bias_v


def _get_program(want_bias_v):
    key = ("prog", want_bias_v)
    if key not in _CACHE:
        _CACHE[key] = _build_program(want_bias_v)
    return _CACHE[key]


def run(inputs, trace=False, trace_kwargs=None):
    """Build + run on all 8 cores. Returns (full_out, BassKernelResults)."""
    from concourse.bass_utils import run_bass_kernel_spmd

    in_maps, want_bias_v = _prep_inputs(**inputs)
    nc = _get_program(want_bias_v)
    kw = {}
    if trace:
        kw["trace"] = True
        if trace_kwargs:
            kw["trace_kwargs"] = trace_kwargs
    res = run_bass_kernel_spmd(nc, in_maps, list(range(NCORES)), **kw)
    out = np.concatenate([res.results[i]["out"] for i in range(NCORES)], axis=0)
    return out.reshape(B, C, HW, HW).astype(np.float32), res


def kernel(**inputs):
    out, _ = run(inputs, trace=False)
    return out
